# revision 60
# baseline (speedup 1.0000x reference)
"""Trainium2 Bass kernel for nn_DecoderLayerWithMOE (attention + dense MoE + FFN layer).

Sharding: 8 cores, zero collectives. Core c owns (batch b = c//2, s-half = c%2)
-> 1024 tokens. Each core computes K/V over the full sequence of its batch,
then attention / MoE / FFN fully token-parallel. Host does slicing, weight
transposes (to bf16), and the final gather. Host orders each core's sequence
so its own tokens are the first T columns.

On-chip: activations feature-major (features on partitions, tokens free).
QKV/attention run in bf16 (activations+weights); accumulation and the
residual stream stay fp32. Q/K/V stay SBUF-resident (no DRAM round-trip).
Attention computes both heads' scores of a pair into one 2-bank PSUM tile and
runs a single Exp per key-tile (Activation engine is the attention
bottleneck). Per-token broadcasts (softmax denominators, layernorm rows,
gate rows) run on the otherwise-idle GPSIMD engine via partition_broadcast
instead of PE ones-matmuls. Layernorm is software-pipelined: stats (PE ones
matmuls) for all chunks first, then per-chunk scalar chains, then per-tile
applies that the next phase's matmuls consume incrementally.
"""

import os
from contextlib import ExitStack

import numpy as np

# Full problem dims
S, B, D, H, E = 2048, 4, 1024, 16, 8
HD = D // H
F = 4 * D
NCORES = 8
P = 128
EPS = 1e-5


class Cfg:
    def __init__(self, D, Skv, T, H, E, F):
        self.D, self.Skv, self.T, self.H, self.E, self.F = D, Skv, T, H, E, F
        self.DT = D // P          # feature tiles
        self.KT = Skv // P        # key-token tiles
        self.CH = min(512, T)     # token chunk (moving N)
        self.NCH = T // self.CH
        self.SKC = Skv // self.CH
        self.FT = F // P
        assert H * 64 == D and F % (4 * P) == 0


FULL_CFG = Cfg(D=D, Skv=S, T=S * B // NCORES, H=H, E=E, F=F)


def build_program(cfg):
    import concourse.bacc as bacc
    import concourse.tile as tile
    import concourse.mybir as mybir

    f32 = mybir.dt.float32
    f32r = mybir.dt.float32r
    bf16 = mybir.dt.bfloat16
    Al = mybir.AluOpType
    Af = mybir.ActivationFunctionType

    DT, KT, CH, NCH, SKC, FT = cfg.DT, cfg.KT, cfg.CH, cfg.NCH, cfg.SKC, cfg.FT
    Dd, Skv, T, Hh, Ee, Ff = cfg.D, cfg.Skv, cfg.T, cfg.H, cfg.E, cfg.F

    nc = bacc.Bacc("TRN2", target_bir_lowering=False, debug=False,
                   num_devices=NCORES)

    def din(name, shape, dt=f32):
        return nc.dram_tensor(name, list(shape), dt, kind="ExternalInput")

    # Weights arrive pre-tiled from the host: [nblk, P, ni*width] with each
    # partition's row fully contiguous in DRAM -> one descriptor-efficient
    # DMA per weight tile (contiguous runs >= 512B avoid the RMW penalty).
    xtb = din("xtb", (Dd, Skv), bf16)      # x^T, own tokens first, bf16
    xr_d = din("xr", (Dd, cfg.T))          # x^T own tokens, f32 (residual)
    wqP = din("wqP", (DT, P, DT * P), bf16)
    wkP = din("wkP", (DT, P, DT * P), bf16)
    wvP = din("wvP", (2, P, DT * (Dd // 2)), bf16)
    woP = din("woP", (DT, P, DT * P), bf16)
    gwP = din("gwP", (P, DT * Ee))  # f32: matmuls against f32r res
    ewP = din("ewP", (Ee * DT, P, DT * P), bf16)
    w1P = din("w1P", (FT, P, DT * P), bf16)
    w2P = din("w2P", (4 * DT, P, (FT // 4) * P), bf16)
    bqp = din("bqp", (P, DT)); bkp = din("bkp", (P, DT))
    bvb_d = din("bvb", (P, Dd)); bop = din("bop", (P, DT))
    gb8_d = din("gb8", (Ee, 1)); gam8_d = din("gam8", (Ee, 1))
    ebp = din("ebp", (P, Ee * DT))
    b1p = din("b1p", (P, FT)); b2p = din("b2p", (P, DT))
    gpa = [din(f"gpa{i}", (P, DT)) for i in range(3)]
    bpa = [din(f"bpa{i}", (P, DT)) for i in range(3)]
    out_d = nc.dram_tensor("out", [Dd, T], f32, kind="ExternalOutput")

    def r(ap):  # f32r view of a dram fp32 AP
        return ap.bitcast(f32r)

    def wload(pool, tag, bufs, name, pret, blk, width_total, dt=bf16):
        """Load pre-tiled weight block `blk` ([P, width_total]) in one DMA."""
        t = pool.tile([P, width_total], dt, tag=tag, bufs=bufs, name=name)
        nc.sync.dma_start(t[:], pret[blk])
        return t

    with ExitStack() as top:
        top.enter_context(nc.allow_low_precision(
            reason="bf16 matmuls within tolerance; fp32 accumulation"))
        tc = top.enter_context(tile.TileContext(nc))
        pers = top.enter_context(tc.tile_pool(name="pers", bufs=1))
        pmm = top.enter_context(tc.tile_pool(name="pmm", bufs=1, space="PSUM"))
        psm = top.enter_context(tc.tile_pool(name="psm", bufs=1))

        def mmtile(name, wide=False):
            return pmm.tile([P, 2 * CH if wide else CH], f32, tag="mm",
                            bufs=3, name=name)

        # ---------- persistent small tensors ----------
        # Tiles created up front; DMAs deferred until after the xt loads so
        # tiny transfers don't head-block the startup-critical queues.
        ones_m = pers.tile([P, P], f32, name="ones_m")
        nc.vector.memset(ones_m[:], 1.0)
        ones_col = pers.tile([P, 1], f32r, name="ones_col")
        nc.vector.tensor_copy(ones_col[:], ones_m[:, 0:1])
        bq_t = pers.tile([P, DT], f32, name="bq_t")
        bk_t = pers.tile([P, DT], f32, name="bk_t")
        bo_t = pers.tile([P, DT], f32, name="bo_t")
        gb8_t = pers.tile([P, 1], f32, name="gb8_t")
        gam8_t = pers.tile([P, 1], f32, name="gam8_t")
        eb_t = pers.tile([P, Ee * DT], f32, name="eb_t")
        b1_t = pers.tile([P, FT], f32, name="b1_t")
        b2_t = pers.tile([P, DT], f32, name="b2_t")
        gp_t = [pers.tile([P, DT], f32, name=f"gp_t{i}") for i in range(3)]
        bp_t = [pers.tile([P, DT], f32, name=f"bp_t{i}") for i in range(3)]
        x2_t = [pers.tile([P, T], bf16, name=f"x2_{i}") for i in range(DT)]

        def load_pers():
            nc.sync.dma_start(bo_t[:], bop[:, :])
            nc.sync.dma_start(gb8_t[0:Ee, :], gb8_d[:, :])
            nc.sync.dma_start(gam8_t[0:Ee, :], gam8_d[:, :])
            nc.sync.dma_start(eb_t[:], ebp[:, :])
            nc.sync.dma_start(b1_t[:], b1p[:, :])
            nc.sync.dma_start(b2_t[:], b2p[:, :])
            for i in range(3):
                nc.sync.dma_start(gp_t[i][:], gpa[i][:, :])
                nc.sync.dma_start(bp_t[i][:], bpa[i][:, :])

        # ---------- pipelined layernorm over DT tiles of [P, T] ----------
        # Split into per-chunk pieces so phases can interleave emission:
        # stats (PE ones-matmuls + Act squares) -> chain (small DVE/Act) +
        # broadcasts (Pool) -> apply (DVE mult/add + Act scale&bias).
        # dst tiles may be a different dtype (bf16 for matmul operands).
        def ln_stats(src, c, ptr):
            cs = slice(c * CH, (c + 1) * CH)
            sum_ps = mmtile("ln_sum")
            sq_ps = mmtile("ln_sq")
            for i in range(DT):
                sq = psm.tile([P, CH], f32r, tag="lnsq", bufs=2,
                              name="ln_sqt")
                nc.scalar.activation(sq[:], src[i][:, cs].bitcast(f32),
                                     Af.Square)
                nc.tensor.matmul(sum_ps[0:1, :], ones_col[:, :],
                                 src[i][:, cs],
                                 start=(i == 0), stop=(i == DT - 1))
                nc.tensor.matmul(sq_ps[0:1, :], ones_col[:, :], sq[:],
                                 start=(i == 0), stop=(i == DT - 1))
            return (sum_ps, sq_ps)

        def ln_bcast(stats, ptr, chain_on_act=False):
            sum_ps, sq_ps = stats
            invD = 1.0 / Dd
            mu = psm.tile([1, CH], f32, tag="mu", bufs=1, name="ln_mu")
            var = psm.tile([1, CH], f32, tag="var", bufs=1, name="ln_var")
            if chain_on_act:
                # consume the stats PSUM on Act: frees the matmul
                # accumulator banks early when DVE is backlogged
                nc.scalar.activation(mu[:], sum_ps[0:1, :], Af.Copy,
                                     scale=invD)
                v0 = psm.tile([1, CH], f32, tag="tmp", bufs=1, name="ln_v0")
                nc.scalar.activation(v0[:], sq_ps[0:1, :], Af.Copy,
                                     scale=invD, bias=EPS)
                nc.scalar.activation(var[:], mu[:], Af.Square)
                nc.vector.tensor_tensor(var[:], v0[:], var[:],
                                        op=Al.subtract)
            else:
                nc.vector.tensor_scalar_mul(mu[:], sum_ps[0:1, :], invD)
                # var = (sq/D + eps) - mu^2
                tmp = psm.tile([1, CH], f32, tag="tmp", bufs=1, name="ln_tmp")
                nc.vector.tensor_tensor(tmp[:], mu[:], mu[:], op=Al.mult)
                nc.vector.tensor_scalar(var[:], sq_ps[0:1, :], invD, EPS,
                                        op0=Al.mult, op1=Al.add)
                nc.vector.tensor_tensor(var[:], var[:], tmp[:],
                                        op=Al.subtract)
            nc.scalar.sqrt(var[:], var[:])
            srow = psm.tile([1, CH], f32, tag="srow", bufs=2, name="ln_srow")
            nc.vector.reciprocal(srow[:], var[:])
            nms = psm.tile([1, CH], f32, tag="nms", bufs=2, name="ln_nms")
            nc.vector.scalar_tensor_tensor(nms[:], mu[:], -1.0, srow[:],
                                           op0=Al.mult, op1=Al.mult)
            s_b = psm.tile([P, CH], f32, tag="lnsb", bufs=2, name="ln_s_b")
            nc.gpsimd.partition_broadcast(s_b[:], srow[:], channels=P)
            nms_b = psm.tile([P, CH], f32, tag="lnnb", bufs=2,
                             name="ln_nms_b")
            nc.gpsimd.partition_broadcast(nms_b[:], nms[:], channels=P)
            return (s_b, nms_b)

        def ln_apply(src, dst, ln_idx, c, bc, ptr, pool_tiles=0,
                     act_step3=True, two_phase=False):
            gpx, bpx = gp_t[ln_idx], bp_t[ln_idx]
            cs = slice(c * CH, (c + 1) * CH)
            s_b, nms_b = bc

            def eng(i):
                # tiles assigned to Pool run their elementwise ops there,
                # in parallel with DVE handling the other tiles
                return nc.gpsimd if i >= DT - pool_tiles else nc.vector

            if two_phase:
                # all src reads first: lets the enclosing pool close (and
                # the next pool's region become writable) as early as
                # possible
                zs = [psm.tile([P, CH], bf16, tag="lnz8", bufs=DT,
                               name="ln_z8") for _ in range(DT)]
                for i in range(DT):
                    eng(i).tensor_tensor(zs[i][:], src[i][:, cs].bitcast(f32),
                                         s_b[:], op=Al.mult)
            else:
                zs = [None] * DT
            for i in range(DT):
                if two_phase:
                    z = zs[i]
                else:
                    if i >= DT - pool_tiles:
                        z = pwf.tile([P, CH], f32, tag="lnzp", bufs=2,
                                     name="ln_zp")
                    else:
                        z = psm.tile([P, CH], f32, tag="lnz", bufs=2,
                                     name="ln_z")
                    eng(i).tensor_tensor(z[:], src[i][:, cs].bitcast(f32),
                                         s_b[:], op=Al.mult)
                eng(i).tensor_tensor(z[:], z[:], nms_b[:], op=Al.add)
                if act_step3:
                    nc.scalar.activation(dst[i][:, cs], z[:], Af.Identity,
                                         bias=bpx[:, i:i + 1],
                                         scale=gpx[:, i:i + 1])
                else:
                    # keep Act free (it would head-block the next phase's
                    # relu queue); 2-scalar tensor_scalar runs at 2x on DVE
                    eng(i).tensor_scalar(dst[i][:, cs], z[:],
                                         gpx[:, i:i + 1], bpx[:, i:i + 1],
                                         op0=Al.mult, op1=Al.add)

        def layer_norm(src, dst, ln_idx, ptr, pool_tiles=0):
            stats = [ln_stats(src, c, ptr) for c in range(NCH)]
            bcs = [ln_bcast(stats[c], ptr) for c in range(NCH)]
            for c in range(NCH):
                ln_apply(src, dst, ln_idx, c, bcs[c], ptr,
                         pool_tiles=pool_tiles)

        # ================= Phase 1: QKV (bf16, SBUF-resident) ============
        es_qkv = ExitStack()
        pqv = es_qkv.enter_context(tc.tile_pool(name="pqv", bufs=1))
        q_sb = [pqv.tile([P, T], bf16, name=f"q_sb{j}") for j in range(DT)]
        k_sb = [pqv.tile([P, Skv], bf16, name=f"k_sb{j}") for j in range(DT)]
        # v_sb layout: [P keys, H, KT, 65] (64 features + ones col per head)
        v_sb = pqv.tile([P, Hh * KT * 65], bf16, name="v_sb")
        v4 = v_sb[:].rearrange("p (h k c) -> p h k c", k=KT, c=65)
        nc.vector.memset(v4[:, :, :, 64:65], 1.0)

        with tc.tile_pool(name="pxt", bufs=1) as pxt:
            xt_t = [pxt.tile([P, Skv], bf16, name=f"xt{i}")
                    for i in range(DT)]
            bvb = pxt.tile([P, Dd], f32, name="bvb")

            # V pool opened first so wv(oc=0) streams in during the QK loop
            HPC = CH // 64  # heads per o-chunk
            with tc.tile_pool(name="pvv", bufs=1) as pvv:
                with tc.tile_pool(name="pqk", bufs=1) as pqk:
                    # j=0 weights and their biases first, then the x
                    # tiles; everything non-critical goes behind j=1's loads
                    wq0 = wload(pqk, "wq", 3, "wq0", wqP, 0, DT * P)
                    wk0 = wload(pqk, "wk", 3, "wk0", wkP, 0, DT * P)
                    nc.sync.dma_start(bq_t[:], bqp[:, :])
                    nc.sync.dma_start(bk_t[:], bkp[:, :])
                    for i in range(DT):
                        nc.sync.dma_start(xt_t[i][:],
                                          xtb[i * P:(i + 1) * P, :])
                    wv0 = None
                    for j in range(DT):
                        wq = wq0 if j == 0 else wload(
                            pqk, "wq", 3, f"wq{j}", wqP, j, DT * P)
                        wk = wk0 if j == 0 else wload(
                            pqk, "wk", 3, f"wk{j}", wkP, j, DT * P)
                        if j == 1:
                            nc.sync.dma_start(bvb[:], bvb_d[:, :])
                            load_pers()
                            wv0 = wload(pvv, "wv", 2, "wv0", wvP, 0,
                                        DT * (Dd // 2))
                        for c in range(NCH):
                            ps = mmtile("q_ps")
                            for i in range(DT):
                                nc.tensor.matmul(
                                    ps[:, :], wq[:, i * P:(i + 1) * P],
                                    xt_t[i][:, c * CH:(c + 1) * CH],
                                    start=(i == 0), stop=(i == DT - 1))
                            nc.vector.tensor_scalar_add(
                                q_sb[j][:, c * CH:(c + 1) * CH], ps[:, :],
                                bq_t[:, j:j + 1])
                        for c in range(SKC):
                            ps = mmtile("k_ps")
                            for i in range(DT):
                                nc.tensor.matmul(
                                    ps[:, :], wk[:, i * P:(i + 1) * P],
                                    xt_t[i][:, c * CH:(c + 1) * CH],
                                    start=(i == 0), stop=(i == DT - 1))
                            nc.vector.tensor_scalar_add(
                                k_sb[j][:, c * CH:(c + 1) * CH], ps[:, :],
                                bk_t[:, j:j + 1])

                # V (activation-stationary, weight-moving) -> v_sb augmented
                for oc in range(Dd // CH):
                    wv = wv0 if oc == 0 else wload(
                        pvv, "wv", 2, f"wv{oc}", wvP, oc, DT * (Dd // 2))
                    for tt in range(KT):
                        ps = mmtile("v_ps")
                        for i in range(DT):
                            nc.tensor.matmul(ps[:, :],
                                             xt_t[i][:, tt * P:(tt + 1) * P],
                                             wv[:, i * CH:(i + 1) * CH],
                                             start=(i == 0), stop=(i == DT - 1))
                        h0 = oc * HPC
                        nc.vector.tensor_tensor(
                            v4[:, h0:h0 + HPC, tt, 0:64],
                            ps[:, :].rearrange("p (h c) -> p h c", c=64),
                            bvb[:, oc * CH:(oc + 1) * CH].rearrange(
                                "p (h c) -> p h c", c=64),
                            op=Al.add)

        # ============ Phase 2: attention ============
        es_attn = ExitStack()
        pattn = es_attn.enter_context(
            tc.tile_pool(name="pattn", bufs=1, side="right"))
        ctx_t = [pattn.tile([P, T], bf16, tag=f"ctx{j}", name=f"ctx{j}")
                 for j in range(DT)]
        # prefetch the full out-proj weight during attention (16KB)
        wo_all = pattn.tile([P, DT * DT * P], bf16, name="wo_all")
        for o in range(DT):
            nc.sync.dma_start(wo_all[:, o * DT * P:(o + 1) * DT * P],
                              woP[o])
        for j in range(Hh // 2):
            for c in range(NCH):
                cps = [pmm.tile([P, CH], f32, tag="ctx", bufs=2,
                                name=f"ctx_ps{half}") for half in (0, 1)]
                for kt in range(KT):
                    sps = mmtile("s_ps", wide=True)
                    for half in (0, 1):
                        hb = half * 64
                        nc.tensor.matmul(
                            sps[:, half * CH:(half + 1) * CH],
                            k_sb[j][hb:hb + 64, kt * P:(kt + 1) * P],
                            q_sb[j][hb:hb + 64, c * CH:(c + 1) * CH],
                            start=True, stop=True)
                    pt = pattn.tile([P, 2 * CH], bf16, tag="pt", bufs=2,
                                    name="p_t")
                    nc.scalar.activation(pt[:], sps[:, :], Af.Exp, scale=0.125)
                    for half in (0, 1):
                        nc.tensor.matmul(cps[half][0:65, :],
                                         v4[:, 2 * j + half, kt, :],
                                         pt[:, half * CH:(half + 1) * CH],
                                         start=(kt == 0), stop=(kt == KT - 1))
                for half in (0, 1):
                    rec = psm.tile([1, CH], f32, tag="rec", bufs=2, name="rec")
                    nc.vector.reciprocal(rec[:], cps[half][64:65, :])
                    rb = pattn.tile([64, CH], f32, tag="rb", bufs=2, name="rb")
                    nc.gpsimd.partition_broadcast(rb[:], rec[:], channels=64)
                    nc.vector.tensor_tensor(
                        ctx_t[j][half * 64:half * 64 + 64,
                                 c * CH:(c + 1) * CH],
                        cps[half][0:64, :], rb[:], op=Al.mult)
        es_qkv.close()  # q/k/v freed

        # FFN weight pool opened early: its SBUF region never overlaps the
        # res/x1/acc pools, so FFN weight DMAs have no WAR to wait on.
        es_wf = ExitStack()
        pwf = es_wf.enter_context(tc.tile_pool(name="pwf", bufs=1))

        # ---- out-proj (+ residual & b_out), LN1 interleaved per chunk ----
        es_res = ExitStack()
        pres = es_res.enter_context(tc.tile_pool(name="pres", bufs=1))
        x1_t = [pres.tile([P, T], bf16, name=f"x1_{i}") for i in range(DT)]
        es_resR = ExitStack()
        presR = es_resR.enter_context(tc.tile_pool(name="presR", bufs=1))
        res_t = [presR.tile([P, T], f32r, name=f"res{o}") for o in range(DT)]
        fgs = []
        pgate = pwf.tile([Ee, T], f32r, name="pgate")
        gw = pwf.tile([P, DT * Ee], f32r, name="gw")
        nc.sync.dma_start(gw[:], r(gwP[:, :]))
        for c in range(NCH):
            cs = slice(c * CH, (c + 1) * CH)
            for o in range(DT):
                xrb = pattn.tile([P, CH], f32r, tag="xrb", bufs=2, name="xrb")
                nc.sync.dma_start(xrb[:], r(xr_d[o * P:(o + 1) * P, cs]))
                ps = mmtile("ao_ps")
                for i in range(DT):
                    nc.tensor.matmul(
                        ps[:, :],
                        wo_all[:, (o * DT + i) * P:(o * DT + i + 1) * P],
                        ctx_t[i][:, c * CH:(c + 1) * CH],
                        start=(i == 0), stop=(i == DT - 1))
                nc.vector.scalar_tensor_tensor(
                    res_t[o][:, cs], ps[:, :], bo_t[:, o:o + 1],
                    xrb[:].bitcast(f32), op0=Al.add, op1=Al.add)
            # LN1 for chunk c overlaps out-proj of chunk c+1
            st = ln_stats(res_t, c, pres)
            bc = ln_bcast(st, pres)
            s_b, nms_b = bc
            # gate softmax straight from res: LN is affine, so
            # logits = s_t*(G@res) + nms_t*gamma_e + beta_e with
            # G = gate_w*g1, gamma = gate_w@g1, beta = gate_w@be1+gate_b
            # (host-precomputed) -- no dependency on the x1 applies.
            gl_ps = mmtile("gl_ps")
            for i in range(DT):
                nc.tensor.matmul(gl_ps[0:Ee, :],
                                 gw[:, i * Ee:(i + 1) * Ee],
                                 res_t[i][:, cs],
                                 start=(i == 0), stop=(i == DT - 1))
            egi = pattn.tile([Ee, CH], f32, tag="egi", bufs=1, name="egi")
            nc.vector.tensor_tensor(egi[:], gl_ps[0:Ee, :], s_b[0:Ee, :],
                                    op=Al.mult)
            nc.vector.scalar_tensor_tensor(egi[:], nms_b[0:Ee, :],
                                           gam8_t[0:Ee, :], egi[:],
                                           op0=Al.mult, op1=Al.add)
            eg = pwf.tile([P, CH], f32r, tag="eg", bufs=2, name="eg")
            nc.scalar.activation(eg[0:Ee, :], egi[:], Af.Exp,
                                 bias=gb8_t[0:Ee, :])

            def finish_gate(c=c, cs=cs, eg=eg):
                gs_ps = mmtile("gs_ps")
                nc.tensor.matmul(gs_ps[0:1, :], ones_col[0:Ee, :],
                                 eg[0:Ee, :], start=True, stop=True)
                grec = psm.tile([1, CH], f32, tag="rec", bufs=2, name="grec")
                nc.vector.reciprocal(grec[:], gs_ps[0:1, :])
                gden = pwf.tile([Ee, CH], f32, tag="gden", bufs=1,
                                name="gden")
                nc.gpsimd.partition_broadcast(gden[:], grec[:], channels=Ee)
                nc.vector.tensor_tensor(pgate[0:Ee, cs],
                                        eg[0:Ee, :], gden[:], op=Al.mult)

            fgs.append(finish_gate)  # deferred: PE need not wait on it
            ln_apply(res_t, x1_t, 0, c, bc, pres, pool_tiles=0,
                     act_step3=(c == 0))
        fgs[0]()
        es_attn.close()  # ctx/pt/wo freed
        es_resR.close()  # res freed (x1 lives on)

        # ============ Phase 3: MoE (chunk-outer), LN2 ============
        es_moe = ExitStack()
        pmoe = es_moe.enter_context(tc.tile_pool(name="pmoe", bufs=1))
        acc_t = [pmoe.tile([P, T], f32r, tag=f"acc{o}", name=f"acc{o}")
                 for o in range(DT)]
        for c in range(NCH):
            cs = slice(c * CH, (c + 1) * CH)
            for e in range(Ee):
                grow = pmoe.tile([1, CH], f32r, tag="grow", bufs=2,
                                 name=f"grow{e}_{c}")
                nc.sync.dma_start(grow[:], pgate[e:e + 1, cs])
                geb = pmoe.tile([P, CH], f32, tag="geb", bufs=3,
                                name=f"ge{e}_{c}")
                nc.gpsimd.partition_broadcast(geb[:], grow[:].bitcast(f32),
                                              channels=P)
                for o in range(DT):
                    we = wload(pwf, "we", 6, f"we{e}_{o}_{c}",
                               ewP, e * DT + o, DT * P)
                    ps = mmtile("moe_ps")
                    for i in range(DT):
                        nc.tensor.matmul(ps[:, :], we[:, i * P:(i + 1) * P],
                                         x1_t[i][:, cs],
                                         start=(i == 0), stop=(i == DT - 1))
                    he = pmoe.tile([P, CH], bf16, tag="he", bufs=2, name="he")
                    nc.scalar.activation(he[:], ps[:, :], Af.Relu,
                                         bias=eb_t[:, e * DT + o:e * DT + o + 1])
                    if e == 0:
                        nc.vector.tensor_tensor(
                            acc_t[o][:, cs], he[:], geb[:], op=Al.mult)
                    else:
                        hg = pmoe.tile([P, CH], f32, tag="hg", bufs=2,
                                       name="hg")
                        nc.vector.tensor_tensor(hg[:], he[:], geb[:],
                                                op=Al.mult)
                        nc.vector.tensor_tensor(
                            acc_t[o][:, cs], acc_t[o][:, cs].bitcast(f32),
                            hg[:], op=Al.add)
                    if e == Ee - 1:
                        # resid2 immediately after the last expert's add so
                        # the LN2 stats passes can chase the o-loop
                        eng = nc.vector if o % 2 == 0 else nc.gpsimd
                        eng.tensor_tensor(acc_t[o][:, cs],
                                          acc_t[o][:, cs].bitcast(f32),
                                          x1_t[o][:, cs], op=Al.add)
                    if c == 0 and e == 0 and o == DT - 1 and fgs:
                        fgs[1]()  # chunk-1 gate finisher, deps long ready
                        fgs.clear()
            st2 = ln_stats(acc_t, c, pmoe)
            bc2 = ln_bcast(st2, pmoe)
            ln_apply(acc_t, x2_t, 1, c, bc2, pmoe, pool_tiles=3,
                     act_step3=(c == 0), two_phase=(c == 1))
        es_moe.close()
        es_res.close()

        es_ff = ExitStack()
        pff = es_ff.enter_context(tc.tile_pool(name="pff", bufs=1,
                                               side="right"))

        # ============ Phase 4: FFN + LN3 ============
        fp_t = [pff.tile([P, T], f32r, tag=f"fp{o}", name=f"fp{o}")
                for o in range(DT)]
        FQ = FT // 4  # f-tiles per FFN quarter
        o3_t = [t[:] for t in fp_t]  # in-place, f32r writes (bits == f32)
        for fh in range(4):
            h_t = [pff.tile([P, T], bf16, tag=f"h{i2}", bufs=2,
                            name=f"h{fh}_{i2}") for i2 in range(FQ)]
            # fh==0 runs chunk-outer: chunk-0 matmuls overlap LN2's
            # chunk-1 applies (weights reloaded per chunk, DMA is cheap)
            oc1 = ([(o32, c) for c in range(NCH) for o32 in range(FQ)]
                   if fh == 0 else
                   [(o32, c) for o32 in range(FQ) for c in range(NCH)])
            w1s = {}
            for o32, c in oc1:
                o = fh * FQ + o32
                if o32 not in w1s or fh == 0:
                    w1s[o32] = wload(pwf, "w1", 3, f"w1_{o}_{c}",
                                     w1P, o, DT * P)
                w1 = w1s[o32]
                ps = mmtile("ff1_ps")
                for i in range(DT):
                    nc.tensor.matmul(ps[:, :], w1[:, i * P:(i + 1) * P],
                                     x2_t[i][:, c * CH:(c + 1) * CH],
                                     start=(i == 0), stop=(i == DT - 1))
                nc.scalar.activation(h_t[o32][:, c * CH:(c + 1) * CH],
                                     ps[:, :], Af.Relu,
                                     bias=b1_t[:, o:o + 1])
            # last quarter runs chunk-outer with LN3 chunk c emitted inline
            # so LN3(c0)'s DVE work overlaps ff2(c1) on the PE
            oc_list = ([(o, c) for o in range(DT) for c in range(NCH)]
                       if fh < 3 else
                       [(o, c) for c in range(NCH) for o in range(DT)])
            w2s = {}
            w2tag, w2bufs = ("w2", 3) if fh < 3 else ("w2l", DT)
            for o, c in oc_list:
                if o not in w2s:
                    w2s[o] = wload(pwf, w2tag, w2bufs, f"w2_{fh}_{o}",
                                   w2P, fh * DT + o, FQ * P)
                w2 = w2s[o]
                ps = mmtile("ff2_ps")
                for i2 in range(FQ):
                    nc.tensor.matmul(ps[:, :], w2[:, i2 * P:(i2 + 1) * P],
                                     h_t[i2][:, c * CH:(c + 1) * CH],
                                     start=(i2 == 0), stop=(i2 == FQ - 1))
                cs = slice(c * CH, (c + 1) * CH)
                if fh == 0:
                    # fp = (ps + b2) + x2  (b2 folded in here so fh==3 is a
                    # plain add)
                    nc.vector.scalar_tensor_tensor(
                        fp_t[o][:, cs], ps[:, :], b2_t[:, o:o + 1],
                        x2_t[o][:, cs], op0=Al.add, op1=Al.add)
                else:
                    nc.vector.tensor_tensor(fp_t[o][:, cs],
                                            fp_t[o][:, cs].bitcast(f32),
                                            ps[:, :], op=Al.add)
                if fh == 3 and o == DT - 1:
                    # LN3 chunk c: stats/chain/apply/out, overlapping the
                    # other chunk's ff2 on the PE
                    st = ln_stats(fp_t, c, pff)
                    bc = ln_bcast(st, pff)
                    ln_apply(fp_t, o3_t, 2, c, bc, pff, pool_tiles=3,
                             act_step3=False)
                    for oo in range(DT):
                        nc.sync.dma_start(
                            out_d[oo * P:(oo + 1) * P, cs],
                            o3_t[oo][:, cs].bitcast(f32))
        es_ff.close()
        es_wf.close()

    nc.compile()
    return nc


# ====================== host side ======================

def _pack_col(v, nt):
    # (nt*128,) -> (128, nt) partition-major
    return np.ascontiguousarray(np.asarray(v, np.float32).reshape(nt, P).T)


def _pret(W, ni, width):
    """[ni*P, nblk*width] -> [nblk, P, ni*width] pre-tiled weight blocks."""
    K, M = W.shape
    nblk = M // width
    return np.ascontiguousarray(
        W.reshape(ni, P, nblk, width).transpose(2, 1, 0, 3)
        .reshape(nblk, P, ni * width))


def make_weight_maps(w_in, b_in, w_out, b_out, gate_w, gate_b, exp_w, exp_b,
                     ffn_w1, ffn_b1, ffn_w2, ffn_b2, g1, be1, g2, be2, g3, be3,
                     cfg):
    import ml_dtypes
    Dd, Ee, FT, DT_ = cfg.D, cfg.E, cfg.FT, cfg.DT
    Ff = cfg.F
    FQ = FT // 4
    f = np.float32
    bf = ml_dtypes.bfloat16
    ct = np.ascontiguousarray

    def tb(a):  # transpose + bf16
        return ct(np.asarray(a, f).T.astype(bf))

    w_in = np.asarray(w_in, f)
    ewT = np.asarray(exp_w, f).transpose(0, 2, 1).astype(bf)
    w2T = tb(ffn_w2)
    m = {
        "wqP": _pret(tb(w_in[0:Dd]), DT_, P),
        "wkP": _pret(tb(w_in[Dd:2 * Dd]), DT_, P),
        "wvP": _pret(tb(w_in[2 * Dd:3 * Dd]), DT_, Dd // 2),
        "woP": _pret(tb(w_out), DT_, P),
        "gwP": _pret(ct((np.asarray(gate_w, f)
                         * np.asarray(g1, f)[None, :]).T), DT_, Ee)[0],
        "ewP": np.concatenate([_pret(ewT[e], DT_, P) for e in range(Ee)]),
        "w1P": _pret(tb(ffn_w1), DT_, P),
        "w2P": np.concatenate(
            [_pret(w2T[fh * FQ * P:(fh + 1) * FQ * P], FQ, P)
             for fh in range(4)]),
        "bqp": _pack_col(np.asarray(b_in, f)[0:Dd], DT_),
        "bkp": _pack_col(np.asarray(b_in, f)[Dd:2 * Dd], DT_),
        "bvb": ct(np.broadcast_to(np.asarray(b_in, f)[2 * Dd:3 * Dd], (P, Dd))),
        "bop": _pack_col(b_out, DT_),
        "gb8": (np.asarray(gate_w, f) @ np.asarray(be1, f)
                + np.asarray(gate_b, f)).reshape(Ee, 1),
        "gam8": (np.asarray(gate_w, f) @ np.asarray(g1, f)).reshape(Ee, 1),
        "ebp": ct(np.asarray(exp_b, f).reshape(Ee * DT_, P).T),
        "b1p": _pack_col(ffn_b1, FT),
        "b2p": _pack_col(ffn_b2, DT_),
        "gpa0": _pack_col(g1, DT_), "bpa0": _pack_col(be1, DT_),
        "gpa1": _pack_col(g2, DT_), "bpa1": _pack_col(be2, DT_),
        "gpa2": _pack_col(g3, DT_), "bpa2": _pack_col(be3, DT_),
    }
    return m


_NC_CACHE = {}


def kernel(x, w_in, b_in, w_out, b_out, gate_w, gate_b, exp_w, exp_b,
           ffn_w1, ffn_b1, ffn_w2, ffn_b2, g1, be1, g2, be2, g3, be3):
    import ml_dtypes
    from concourse.bass_utils import run_bass_kernel_spmd

    cfg = FULL_CFG
    bf = ml_dtypes.bfloat16
    x = np.asarray(x, np.float32)
    wm = make_weight_maps(w_in, b_in, w_out, b_out, gate_w, gate_b, exp_w,
                          exp_b, ffn_w1, ffn_b1, ffn_w2, ffn_b2,
                          g1, be1, g2, be2, g3, be3, cfg)
    Th = cfg.T  # tokens per core (one s-half of one batch)
    in_maps = []
    for c in range(NCORES):
        b, half = c // 2, c % 2
        xb = x[:, b, :]                      # (S, D)
        own = xb[half * Th:(half + 1) * Th]  # (T, D)
        other = xb[(1 - half) * Th:(2 - half) * Th]
        xt_c = np.concatenate([own, other], axis=0).T  # (D, Skv), own first
        in_maps.append({**wm,
                        "xtb": np.ascontiguousarray(xt_c.astype(bf)),
                        "xr": np.ascontiguousarray(own.T)})

    if "nc" not in _NC_CACHE:
        _NC_CACHE["nc"] = build_program(cfg)
    nc = _NC_CACHE["nc"]

    trace = bool(int(os.environ.get("KERNEL_TRACE", "0")))
    last_exc = None
    for attempt in range(3):
        try:
            res = run_bass_kernel_spmd(nc, in_maps, core_ids=list(range(NCORES)),
                                       trace=trace)
            break
        except Exception as e:  # transient axon/NRT hiccups — retry
            last_exc = e
            if attempt == 2:
                raise
    _NC_CACHE["last_results"] = res

    out = np.empty((S, B, D), np.float32)
    for c in range(NCORES):
        b, half = c // 2, c % 2
        out[half * Th:(half + 1) * Th, b, :] = res.results[c]["out"].T
    return out
